# revision 1
# baseline (speedup 1.0000x reference)
"""Trainium2 Bass kernel for a separable 3D Haar DWT (nn_DWT3D).

Problem: x [2, 128, 128, 128, 4] fp32, A [128, 128] (orthonormal Haar
analysis filter bank, 2-tap stride-2). Output: subband concat
[2, 64, 64, 64, 32].

Strategy (8 NeuronCores):
- Data-parallel over (batch, channel): B*C = 8 independent [128,128,128]
  volume transforms, one per core. Host deinterleaves channels on the
  way in and assembles the subband concat on the way out (numpy).
- KEY TRICK: the host lays each volume out with partition index
  p = (i&1, j&1, k>>2) and free index f = (i>>1, j>>1, k&3). The Haar
  i-pass AND j-pass then act entirely on the partition axis, so ONE
  128x128 PE matmul (butterfly_i (x) butterfly_j (x) I_32, 4 nonzeros
  per row, scales h^3 folded in) performs both passes at the cost of a
  single pass. The k-pass pairs differ only in k&1 (free axis) and run
  as DVE add/sub butterflies straight off the DMA'd input, before the
  matmul. The vector engine does ONE pass instead of two.
- Per chunk: DMA-in (sync ring) -> DVE k-butterfly -> PE matmul ->
  ACT drains PSUM -> DMA-out (scalar ring).
"""

import numpy as np

_N = 128
_CORES = 8
# chunk sizes in i-pair units (each = 256 columns); sum to 64.
_MI = [4, 8, 8, 8, 8, 8, 8, 8, 2, 2]

_cache = {}


def _build():
    import concourse.mybir as mybir
    from concourse import bacc
    from concourse.tile import TileContext

    nc = bacc.Bacc("TRN2", target_bir_lowering=False, debug=False,
                   num_devices=_CORES)
    v = nc.dram_tensor("v", [_N, _N * _N], mybir.dt.float32,
                       kind="ExternalInput")
    w = nc.dram_tensor("w", [_N, _N], mybir.dt.float32,
                       kind="ExternalInput")
    y = nc.dram_tensor("y", [_N, _N * _N], mybir.dt.float32,
                       kind="ExternalOutput")
    f32 = mybir.dt.float32

    chunks = []
    off = 0
    for mi in _MI:
        chunks.append((off, mi * 256))
        off += mi * 256
    assert off == _N * _N

    with TileContext(nc) as tc:
        with (
            tc.tile_pool(name="wpool", bufs=1) as wpool,
            tc.tile_pool(name="vin", bufs=4) as vpool,
            tc.tile_pool(name="y0", bufs=4) as y0pool,
            tc.tile_pool(name="y1", bufs=4) as y1pool,
            tc.tile_pool(name="ps", bufs=2, space="PSUM") as pspool,
        ):
            wt = wpool.tile([_N, _N], f32)
            nc.scalar.dma_start(out=wt[:], in_=w[:])

            for off, cw in chunks:
                vin = vpool.tile([_N, cw], f32, tag="vin")
                nc.sync.dma_start(out=vin[:], in_=v[:, off:off + cw])

                # k-axis butterfly on the free axis: kl = 2*ml + eps,
                # pairs (0,1) and (2,3) -> y0 free (a, lhk, ml)
                vv = vin[:].rearrange("p (a k) -> p a k", k=4)
                y0 = y0pool.tile([_N, cw], f32, tag="y0")
                y0v = y0[:].rearrange("p (a l m) -> p a l m", l=2, m=2)
                nc.vector.tensor_add(out=y0v[:, :, 0, :],
                                     in0=vv[:, :, 0:4:2],
                                     in1=vv[:, :, 1:4:2])
                nc.vector.tensor_sub(out=y0v[:, :, 1, :],
                                     in0=vv[:, :, 1:4:2],
                                     in1=vv[:, :, 0:4:2])

                # i-pass and j-pass together: one matmul over partitions
                ps = pspool.tile([_N, cw], f32, tag="ps")
                for m in range(0, cw, 512):
                    n = min(512, cw - m)
                    nc.tensor.matmul(ps[:, m:m + n], wt[:], y0[:, m:m + n],
                                     start=True, stop=True)

                # ACT drains PSUM -> SBUF, then out-DMA on the scalar ring
                y1 = y1pool.tile([_N, cw], f32, tag="y1")
                nc.scalar.copy(out=y1[:], in_=ps[:])
                nc.scalar.dma_start(out=y[:, off:off + cw], in_=y1[:])

    nc.compile()
    return nc


def _get_nc():
    if "nc" not in _cache:
        _cache["nc"] = _build()
    return _cache["nc"]


def _haar_structure_ok(A):
    """A must be the 2-tap stride-2 filter bank with taps (h, h) lowpass /
    (-h, h) highpass, which is what the butterflies hardcode."""
    if A.shape != (_N, _N):
        return False
    h = A[0, 0]
    if not np.isfinite(h) or abs(h) < 1e-8:
        return False
    expect = np.zeros((_N, _N), dtype=np.float32)
    for i in range(_N // 2):
        expect[i, 2 * i] = h
        expect[i, 2 * i + 1] = h
        expect[_N // 2 + i, 2 * i] = -h
        expect[_N // 2 + i, 2 * i + 1] = h
    return bool(np.allclose(A, expect, rtol=1e-5, atol=1e-7))


def _reference_host(x, A):
    """Generic numpy fallback (slow) for non-Haar A."""
    y = np.einsum("ai,nijkc->najkc", A, x, optimize=True)
    y = np.einsum("bj,najkc->nabkc", A, y, optimize=True)
    y = np.einsum("dk,nabkc->nabdc", A, y, optimize=True)
    return np.moveaxis(y, -1, 1)


def _assemble(y_full, B, C):
    """Slice transformed volumes y_full [B, C, 128,128,128] into the
    reference's subband concat [B, 64, 64, 64, 8*C] (incl. the duplicated
    HHH octant the reference produces)."""
    L, H = slice(0, 64), slice(64, 128)
    bands = [(L, L, L), (H, L, L), (L, H, L), (H, H, L),
             (L, L, H), (H, H, H), (L, H, H), (H, H, H)]
    out = np.empty((B, 64, 64, 64, 8 * C), dtype=np.float32)
    for s, (sa, sb, sd) in enumerate(bands):
        out[..., s * C:(s + 1) * C] = np.moveaxis(y_full[:, :, sa, sb, sd], 1, -1)
    return out


def kernel(x, A):
    from concourse.bass_utils import run_bass_kernel_spmd

    x = np.asarray(x, dtype=np.float32)
    A = np.asarray(A, dtype=np.float32)
    B, _, _, _, C = x.shape
    assert (B, C) == (2, 4) and x.shape[1:4] == (_N, _N, _N)

    if not _haar_structure_ok(A):
        return _assemble(_reference_host(x, A), B, C)

    h = float(A[0, 0])
    # W maps partition (ei, ej, kh) -> (lhi, lhj, kh):
    # butterfly_i (x) butterfly_j (x) I_32, with all three passes' filter
    # scale (h^3) folded in. lhsT = W.T.
    sgn = np.array([[1.0, 1.0], [-1.0, 1.0]], dtype=np.float32)
    Wm = np.zeros((_N, _N), dtype=np.float32)
    h3 = np.float32(h ** 3)
    for lhi in range(2):
        for lhj in range(2):
            for ei in range(2):
                for ej in range(2):
                    c = h3 * sgn[lhi, ei] * sgn[lhj, ej]
                    for kh in range(32):
                        Wm[lhi * 64 + lhj * 32 + kh,
                           ei * 64 + ej * 32 + kh] = c
    wT = np.ascontiguousarray(Wm.T)

    # Host layout: p = (i&1, j&1, k>>2), f = (i>>1, j>>1, k&3)
    xs = np.transpose(x, (0, 4, 1, 2, 3))                 # [B, C, i, j, k]
    t = xs.reshape(_CORES, 64, 2, 64, 2, 32, 4)           # [g,mi,ei,mj,ej,kh,kl]
    t = np.ascontiguousarray(t.transpose(0, 2, 4, 5, 1, 3, 6))
    V = t.reshape(_CORES, _N, _N * _N)

    in_maps = [{"v": V[g], "w": wT} for g in range(_CORES)]
    nc = _get_nc()
    res = run_bass_kernel_spmd(nc, in_maps, list(range(_CORES)))

    # Output rows (lhi, lhj, kh), free (mi, mj, lhk, ml):
    # a = 64*lhi + mi, b = 64*lhj + mj, d = 64*lhk + 2*kh + ml.
    y_full = np.empty((B, C, _N, _N, _N), dtype=np.float32)
    for g in range(_CORES):
        z = res.results[g]["y"].reshape(2, 2, 32, 64, 64, 2, 2)
        vol = z.transpose(0, 3, 1, 4, 5, 2, 6).reshape(_N, _N, _N)
        y_full[g // C, g % C] = vol
    return _assemble(y_full, B, C)



# revision 2
# speedup vs baseline: 1.6868x; 1.6868x over previous
"""Trainium2 Bass kernel for a separable 3D Haar DWT (nn_DWT3D).

Problem: x [2, 128, 128, 128, 4] fp32, A [128, 128] (orthonormal Haar
analysis filter bank, 2-tap stride-2). Output: subband concat
[2, 64, 64, 64, 32].

Strategy (8 NeuronCores):
- Data-parallel over (batch, channel): B*C = 8 independent [128,128,128]
  volume transforms, one per core. Host deinterleaves channels on the
  way in and assembles the subband concat on the way out (numpy).
- KEY TRICK: the host lays each volume out with partition index
  p = (i&1, j&1, k&1, (k>>1)&15) and free index f = (i>>1, j>>1, k>>5).
  ALL THREE Haar butterfly passes then act on the partition axis, so ONE
  128-wide PE matmul (butterfly_i (x) butterfly_j (x) butterfly_k (x)
  I_16, 8 nonzeros per row, h^3 folded in) performs the whole 3D
  transform. No vector-engine pass at all.
- fp16 everywhere off-chip (tolerance is 2e-2; fp16 keeps us ~1e-3):
  halves DMA bytes and runs the PE at 1 cycle/row instead of fp32's
  effective 8 (2 half-speed passes).
- The reference's subband concat never emits the (a=H, b=L, d=H) octant
  (its HLH slice equals HHH), so only 112 of 128 output rows are
  computed, drained, and DMA'd out.
- Per 2048-col chunk: SP issues in-DMA -> 4 PE matmuls (512 cols each,
  one PSUM bank) -> drain PSUM->SBUF fp16 split across ACT and DVE ->
  GpSimd (SWDGE) issues out-DMA.
"""

import numpy as np

_N = 128
_CORES = 8
_F = _N * _N  # 16384 free columns per volume
_CHUNK = 2048
_QROWS = 112  # 7 of 8 output octants * 16 kh rows
# ACT/DVE drain split within a chunk (ACT is ~1.25x faster per column)
_ACT_COLS = 1152

# Octants (lhi, lhj, lhk) in device output row order; (1, 0, 1) omitted.
_OCT = [(0, 0, 0), (1, 0, 0), (0, 1, 0), (1, 1, 0),
        (0, 0, 1), (0, 1, 1), (1, 1, 1)]
# reference band order LLL,LLH,LHL,LHH,HLL,HLH,HHL,HHH -> index into _OCT
_OMAP = [0, 1, 2, 3, 4, 6, 5, 6]

_cache = {}


def _build():
    import concourse.mybir as mybir
    from concourse import bacc
    from concourse.tile import TileContext

    nc = bacc.Bacc("TRN2", target_bir_lowering=False, debug=False,
                   num_devices=_CORES)
    f16 = mybir.dt.float16
    f32 = mybir.dt.float32
    v = nc.dram_tensor("v", [_N, _F], f16, kind="ExternalInput")
    w = nc.dram_tensor("w", [_N, _QROWS], f16, kind="ExternalInput")
    y = nc.dram_tensor("y", [_QROWS, _F], f16, kind="ExternalOutput")

    with TileContext(nc) as tc:
        with (
            tc.tile_pool(name="wpool", bufs=1) as wpool,
            tc.tile_pool(name="vin", bufs=1) as vpool,
            tc.tile_pool(name="yout", bufs=1) as ypool,
            tc.tile_pool(name="ps", bufs=2, space="PSUM") as pspool,
        ):
            wt = wpool.tile([_N, _QROWS], f16)
            nc.scalar.dma_start(out=wt[:], in_=w[:])

            vin = vpool.tile([_N, _F], f16)
            yt = ypool.tile([_QROWS, _F], f16)

            for off in range(0, _F, _CHUNK):
                nc.sync.dma_start(out=vin[:, off:off + _CHUNK],
                                  in_=v[:, off:off + _CHUNK])

                ps = pspool.tile([_QROWS, _CHUNK], f32, tag="ps")
                for m in range(0, _CHUNK, 512):
                    nc.tensor.matmul(ps[:, m:m + 512], wt[:],
                                     vin[:, off + m:off + m + 512],
                                     start=True, stop=True)

                # drain PSUM fp32 -> SBUF fp16, split ACT / DVE
                nc.scalar.copy(out=yt[:, off:off + _ACT_COLS],
                               in_=ps[:, 0:_ACT_COLS])
                nc.vector.tensor_copy(yt[:, off + _ACT_COLS:off + _CHUNK],
                                      ps[:, _ACT_COLS:_CHUNK])

                nc.gpsimd.dma_start(out=y[:, off:off + _CHUNK],
                                    in_=yt[:, off:off + _CHUNK])

    nc.compile()
    return nc


def _get_nc():
    if "nc" not in _cache:
        _cache["nc"] = _build()
    return _cache["nc"]


def _haar_structure_ok(A):
    """A must be the 2-tap stride-2 filter bank with taps (h, h) lowpass /
    (-h, h) highpass, which is what the butterflies hardcode."""
    if A.shape != (_N, _N):
        return False
    h = A[0, 0]
    if not np.isfinite(h) or abs(h) < 1e-8:
        return False
    expect = np.zeros((_N, _N), dtype=np.float32)
    for i in range(_N // 2):
        expect[i, 2 * i] = h
        expect[i, 2 * i + 1] = h
        expect[_N // 2 + i, 2 * i] = -h
        expect[_N // 2 + i, 2 * i + 1] = h
    return bool(np.allclose(A, expect, rtol=1e-5, atol=1e-7))


def _reference_host(x, A):
    """Generic numpy fallback (slow) for non-Haar A."""
    y = np.einsum("ai,nijkc->najkc", A, x, optimize=True)
    y = np.einsum("bj,najkc->nabkc", A, y, optimize=True)
    y = np.einsum("dk,nabkc->nabdc", A, y, optimize=True)
    return np.moveaxis(y, -1, 1)


def _assemble(y_full, B, C):
    """Slice transformed volumes y_full [B, C, 128,128,128] into the
    reference's subband concat [B, 64, 64, 64, 8*C] (incl. the duplicated
    HHH octant the reference produces)."""
    L, H = slice(0, 64), slice(64, 128)
    bands = [(L, L, L), (H, L, L), (L, H, L), (H, H, L),
             (L, L, H), (H, H, H), (L, H, H), (H, H, H)]
    out = np.empty((B, 64, 64, 64, 8 * C), dtype=np.float32)
    for s, (sa, sb, sd) in enumerate(bands):
        out[..., s * C:(s + 1) * C] = np.moveaxis(y_full[:, :, sa, sb, sd], 1, -1)
    return out


def kernel(x, A):
    from concourse.bass_utils import run_bass_kernel_spmd

    x = np.asarray(x, dtype=np.float32)
    A = np.asarray(A, dtype=np.float32)
    B, _, _, _, C = x.shape
    assert (B, C) == (2, 4) and x.shape[1:4] == (_N, _N, _N)

    if not _haar_structure_ok(A):
        return _assemble(_reference_host(x, A), B, C)

    h = float(A[0, 0])
    # W maps partition (ei, ej, ek, kh) -> (octant o, kh): all three
    # butterfly passes at once, h^3 folded in. lhsT = W.T [128, 112].
    sgn = np.array([[1.0, 1.0], [-1.0, 1.0]], dtype=np.float32)
    Wm = np.zeros((_QROWS, _N), dtype=np.float32)
    h3 = np.float32(h ** 3)
    for o, (lhi, lhj, lhk) in enumerate(_OCT):
        for ei in range(2):
            for ej in range(2):
                for ek in range(2):
                    c = h3 * sgn[lhi, ei] * sgn[lhj, ej] * sgn[lhk, ek]
                    for kh in range(16):
                        Wm[o * 16 + kh,
                           ei * 64 + ej * 32 + ek * 16 + kh] = c
    wT = np.ascontiguousarray(Wm.T.astype(np.float16))

    # Host layout: p = (i&1, j&1, k&1, (k>>1)&15), f = (i>>1, j>>1, k>>5)
    xs = np.transpose(x, (0, 4, 1, 2, 3))               # [B, C, i, j, k]
    t = xs.reshape(_CORES, 64, 2, 64, 2, 4, 16, 2)      # g,mi,ei,mj,ej,ml,kh,ek
    t = t.transpose(0, 2, 4, 7, 6, 1, 3, 5)             # g,ei,ej,ek,kh,mi,mj,ml
    V = np.ascontiguousarray(t).astype(np.float16).reshape(_CORES, _N, _F)

    in_maps = [{"v": V[g], "w": wT} for g in range(_CORES)]
    nc = _get_nc()
    res = run_bass_kernel_spmd(nc, in_maps, list(range(_CORES)))

    # Device rows (o, kh), cols (mi, mj, ml). Within-octant output index:
    # a = 64*lhi + mi, b = 64*lhj + mj, d = 64*lhk + 16*ml + kh.
    out = np.empty((B, 64, 64, 64, 8 * C), dtype=np.float32)
    for g in range(_CORES):
        b, c = g // C, g % C
        z = np.asarray(res.results[g]["y"]).reshape(7, 16, 64, 64, 4)
        z = z.transpose(0, 2, 3, 4, 1).astype(np.float32)  # o,mi,mj,ml,kh
        z = z.reshape(7, 64, 64, 64)
        for s in range(8):
            out[b, :, :, :, s * C + c] = z[_OMAP[s]]
    return out
